# revision 1
# baseline (speedup 1.0000x reference)
"""Trainium2 Bass kernel for BasePropagationGraphPositionalEncoding.

Computes, for each batch element b:
    out[b] = (sum_k coefs[k] * gr_kernel[b, k]) @ x[b] / sum_k coefs[k]
with coefs[k] = (1 - EPS)^k, EPS = 0.01, K = 9.

Sharding: batch dim B=8 across the 8 NeuronCores (data parallel, no
cross-core communication). Each core streams its 36 MB of gr_kernel slabs
from HBM (the memory-bound term).

Design (each choice below was trace-driven; see inline comments):
  - All loads are f32 on the HWDGE (sync) ring, batched 6+2+1 per band
    so Tile's 8 round-robin DMA-completion semaphore lanes recycle slowly
    enough to decouple the stream from the consumers (the dominant
    bottleneck in every earlier variant).
  - The weighted k-sum runs as per-slab scale+cast-to-fp16 (ScalarE for 3
    slabs, VectorE tensor_scalar for 6) plus fp16 2x_1P tensor_tensor
    ADDs: ~9.2us/band on VectorE, under the ~11.3us stream cadence
    (a straight fp32 STT chain, the baseline design, is ~13us/band and
    co-bottlenecks with the stream).
  - TensorE transposes and the contraction run single-pass fp16; the
    post-k-sum stage is emitted one iteration late (software pipelining)
    so no cross-band serialized loop forms across engine queues.
  - The 1/sum(coefs) normalization rides free on the ScalarE PSUM->SBUF
    output copy (activation scale). fp16 (not bf16) keeps the total
    quantization error ~6e-4 << the 2e-2 gate.
"""

import sys

if "/opt/trn_rl_repo" not in sys.path:
    sys.path.insert(0, "/opt/trn_rl_repo")

import numpy as np

import concourse.bass as bass
import concourse.mybir as mybir
from concourse import tile
from concourse.bacc import Bacc
from concourse.masks import make_identity
from concourse.bass_utils import run_bass_kernel_spmd

# Problem shapes (hardcoded per the harness contract).
B, K, N, D = 8, 9, 1024, 64
EPS = 0.01
P = 128          # SBUF partitions
NT = N // P      # 8 row/col tiles of the [N, N] kernel

F32 = mybir.dt.float32
F16 = mybir.dt.float16

R = 1.0 - EPS                                  # Horner ratio
S = float(sum(R ** k for k in range(K)))       # sum of coefs


def build_bass() -> bass.Bass:
    # Bacc (not plain Bass): its compile() runs generate_event_semaphores /
    # move_matmul_waits_to_ldweights, splitting multi-semaphore waits that
    # the 64B ISA instructions (single EVENTS slot) cannot carry.
    nc = Bacc()

    x_d = nc.dram_tensor("x_b", (N, D), F32, kind="ExternalInput")
    g_d = nc.dram_tensor("g_b", (K, N, N), F32, kind="ExternalInput")
    o_d = nc.dram_tensor("out_b", (N, D), F32, kind="ExternalOutput")

    with tile.TileContext(nc) as tc:
        with (
            tc.tile_pool(name="consts", bufs=1) as consts,
            tc.tile_pool(name="gr", bufs=3) as gr_pool,
            tc.tile_pool(name="wk", bufs=3) as wk_pool,
            tc.tile_pool(name="wkt", bufs=2) as wkt_pool,
            tc.tile_pool(name="outp", bufs=2) as out_pool,
            tc.tile_pool(name="ps_t", bufs=4, space=bass.MemorySpace.PSUM) as ps_t,
            tc.tile_pool(name="ps_e", bufs=2, space=bass.MemorySpace.PSUM) as ps_e,
        ):
            # Per-band f32 loads, all on the HWDGE (sync) ring: RTL
            # descriptors at ~zero issue cost, the only load path that
            # sustains HBM rate. (Casting f32->fp16 during the DMA was
            # tried and is a dead end: it forces the SWDGE/gpsimd software
            # path, whose Q7 emission caps below HBM rate and whose
            # end-of-program ring drain adds ~14us.)
            #
            # Loads are batched 6+2+1, not 9x1: Tile tracks DMA
            # completion on 8 round-robin semaphore lanes, and a lane
            # cannot be reset for reuse until the consumer ops that waited
            # on its previous value have retired. Nine DMAs/band recycle
            # the lanes every band, coupling the stream to the add chain
            # with <1 band of slack (measured: the sync queue stalled
            # 5.6-8.9us on lane resets every single band). Three DMAs/band
            # give >2 bands of decoupling.
            g_r = g_d.rearrange("k (b p) n -> b p k n", p=P)
            KB0 = 3  # k >= KB0 ride the batched DMA

            def load_band(i):
                tiles = {}
                g_big = gr_pool.tile([P, (K - KB0) * N], F32, tag="gbig", name="g_big")
                nc.sync.dma_start(g_big[:], g_r[i, :, KB0:K, :])
                for k in range(KB0, K):
                    tiles[k] = g_big[:, (k - KB0) * N : (k - KB0 + 1) * N]
                # Low-k loads are [k=2,1] paired + [k=0] single: one
                # fewer semaphore-lane use per band than all-singles (A/B:
                # 4-9.5us faster), while k=0 stays its own DMA so the add
                # chain's final op tracks the last arrival (batching all of
                # k=2..0 as one transfer was slower: the chain then waits
                # on coarse completions at both ends of the band).
                g_21 = gr_pool.tile([P, 2 * N], F32, tag="g21", name="g_21")
                nc.sync.dma_start(g_21[:], g_r[i, :, 1:3, :])
                tiles[2] = g_21[:, N : 2 * N]
                tiles[1] = g_21[:, 0:N]
                g_0 = gr_pool.tile([P, N], F32, tag="gf0", name="gf_0")
                nc.sync.dma_start(g_0[:], g_d[0, i * P : (i + 1) * P, :])
                tiles[0] = g_0[:]
                return tiles

            band_tiles = load_band(0)

            # x rearranged to [p, chunk, d] so chunk c is a [128, 64] tile
            # with the contraction index m = c*128 + p on partitions.
            x_sb = consts.tile([P, NT, D], F16)
            nc.gpsimd.dma_start(x_sb[:], x_d.rearrange("(c p) d -> p c d", p=P))

            # fp16 identity for TensorE transpose. Built by GPSIMD, then
            # copied through VectorE so the first PE transpose waits on a
            # single semaphore (DVE) - Matmult lowering only supports one
            # sync wait.
            ident_raw = consts.tile([P, P], F16)
            make_identity(nc, ident_raw)
            ident = consts.tile([P, P], F16)
            nc.vector.tensor_copy(ident[:], ident_raw[:])


            # Post-k-sum pipeline for one band: transposes, wkT staging,
            # contraction, output. Emitted one iteration LATE (software
            # pipelining): if band i's wkT copies were emitted before band
            # i+1's wk-init on the ScalarE queue, init(i+1) would queue
            # behind copies that wait on band i's full add-chain +
            # transposes -- a serialized ~13.7us/band cross-band loop that
            # throttled the DMA stream to ~360 GB/s. Deferring the tail
            # stage one iteration removes every cross-band gate except the
            # engines' own (sub-budget) throughput.
            def emit_tail(i, wk):
                # Transpose the 8 [128,128] fp16 tiles of wk on TensorE;
                # each chunk staged to SBUF by its own ACT copy.
                wkT_sb = wkt_pool.tile([P, NT, P], F16, name="wkT_sb")
                for c in range(NT):
                    wkT_ps = ps_t.tile([P, P], F16, name="wkT_ps")
                    nc.tensor.transpose(wkT_ps[:], wk[:, c * P : (c + 1) * P], ident[:])
                    nc.scalar.copy(wkT_sb[:, c, :], wkT_ps[:])

                # emb[i-band] = sum_c wk_tile(i,c) @ x_chunk(c), accumulated
                # in PSUM (fp32) over the 8 contraction chunks.
                emb_ps = ps_e.tile([P, D], F32, name="emb_ps")
                for c in range(NT):
                    nc.tensor.matmul(
                        emb_ps[:],
                        wkT_sb[:, c, :],
                        x_sb[:, c, :],
                        start=(c == 0),
                        stop=(c == NT - 1),
                    )

                # PSUM -> SBUF with the 1/S normalization folded into the
                # ACT copy's free scale.
                o_sb = out_pool.tile([P, D], F32, name="o_sb")
                nc.scalar.activation(
                    o_sb[:], emb_ps[:], mybir.ActivationFunctionType.Copy,
                    scale=1.0 / S,
                )
                # Output DMA on the ACT HWDGE ring, NOT sync: an out-DMA on
                # the sync FIFO would block the f32 slab loads queued behind
                # it until this band's whole pipeline finishes (measured:
                # 20us sync-queue stalls, chain starts oscillating).
                # (Batching all 8 outputs into one end-of-kernel DMA was
                # also tried: 134.9us vs 126.5 -- keep per-band outputs.)
                nc.scalar.dma_start(o_d[i * P : (i + 1) * P, :], o_sb[:])

            pending = None
            for i in range(NT):
                g_ts = band_tiles
                if i + 1 < NT:
                    band_tiles = load_band(i + 1)

                # Weighted k-sum, wk = sum_k r^k * g_k, accumulated in fp16
                # (fp16's 10 mantissa bits keep total rel err ~6e-4).
                # Per-slab scale+cast f32->fp16, then a 2x_1P fp16
                # tensor_tensor ADD (0.69us). Three slabs get the
                # scale+cast on ScalarE (k=8 initializes wk directly);
                # six use VectorE tensor_scalar (fp32 2x_2P, 0.59us).
                # DVE total ~9.1us/band, under the ~11.6us stream cadence
                # (a straight fp32 STT chain would be ~11.8 — the
                # baseline's co-bottleneck). Emitted in slab-consumption
                # order (k=8 first) so the chain tracks the DMA stream.
                wk = wk_pool.tile([P, N], F16)
                nc.scalar.activation(
                    wk[:], g_ts[K - 1], mybir.ActivationFunctionType.Copy,
                    scale=R ** (K - 1),
                )
                for k in range(K - 2, -1, -1):
                    gh_k = wk_pool.tile([P, N], F16, tag=f"gh{k}", name=f"gh_{k}")
                    if k >= K - 3:  # k=7,6: scale+cast on ScalarE
                        nc.scalar.activation(
                            gh_k[:], g_ts[k],
                            mybir.ActivationFunctionType.Copy, scale=R ** k,
                        )
                    else:  # k=5..0: scale+cast on VectorE
                        nc.vector.tensor_scalar_mul(gh_k[:], g_ts[k], R ** k)
                    nc.vector.tensor_add(wk[:], wk[:], gh_k[:])

                if pending is not None:
                    emit_tail(*pending)
                pending = (i, wk)

            emit_tail(*pending)

    nc.compile()
    return nc


_NC = None


def _get_nc() -> bass.Bass:
    global _NC
    if _NC is None:
        _NC = build_bass()
    return _NC


def run(x: np.ndarray, gr_kernel: np.ndarray, **spmd_kwargs):
    """Run the SPMD kernel on cores 0-7; returns BassKernelResults."""
    nc = _get_nc()
    in_maps = [
        {
            "x_b": np.ascontiguousarray(x[b], dtype=np.float32),
            "g_b": np.ascontiguousarray(gr_kernel[b], dtype=np.float32),
        }
        for b in range(B)
    ]
    return run_bass_kernel_spmd(nc, in_maps, core_ids=list(range(B)), **spmd_kwargs)


def kernel(x: np.ndarray, gr_kernel: np.ndarray) -> np.ndarray:
    res = run(np.asarray(x), np.asarray(gr_kernel))
    out = np.stack([res.results[b]["out_b"] for b in range(B)], axis=0)
    return out.astype(np.float32, copy=False)


if __name__ == "__main__":
    rng = np.random.default_rng(0)
    x = rng.standard_normal((B, N, D), dtype=np.float32)
    g = rng.standard_normal((B, K, N, N), dtype=np.float32)
    out = kernel(x, g)
    coefs = (1.0 - EPS) ** np.arange(K)
    wk = np.einsum("k,bknm->bnm", coefs, g)
    ref = np.matmul(wk, x) / coefs.sum()
    err = np.linalg.norm(out - ref) / np.linalg.norm(ref)
    print("self-check rel err:", err)



# revision 5
# speedup vs baseline: 1.1689x; 1.1689x over previous
"""Trainium2 Bass kernel for BasePropagationGraphPositionalEncoding.

Computes, for each batch element b:
    out[b] = (sum_k coefs[k] * gr_kernel[b, k]) @ x[b] / sum_k coefs[k]
with coefs[k] = (1 - EPS)^k, EPS = 0.01, K = 9.

Sharding: batch dim B=8 across the 8 NeuronCores (data parallel, no
cross-core communication). Each core streams its 37.75 MB of gr_kernel
slabs from HBM (the memory-bound term; ~410 GB/s/core measured).

Design (trace-driven, v2):
  - All gr_kernel loads are f32 on the HWDGE (sync) ring, 3 DMAs/band of
    3 slabs each (k=6..8 first). Finer than the old 6+2+1 batching so the
    weighted-sum chain starts as soon as the first 1.5 MB lands (was: the
    chain's first op waited on a 3 MB batch = ~8us of dead pipeline fill),
    while staying at 3 DMAs/band so Tile's 8 round-robin DMA-completion
    lanes recycle slowly enough to decouple the stream from the consumers.
  - The weighted k-sum uses the fused scalar_tensor_tensor op
    (wk = (g_k * c_k) + wk): 9 ops per band instead of the old 14
    (6 tensor_scalar muls + 8 adds + 3 ACT scales). It is split
    column-wise: VectorE owns cols [0:VC), GpSimd owns [VC:N) - two
    independent serial chains, no cross-engine dependency. GpSimd
    elementwise runs at ~0.42x roofline, hence the asymmetric split.
  - x is loaded f32 on the ACT (HWDGE) ring and cast to fp16 by one ACT
    copy. The old SWDGE (gpsimd) cast-during-DMA path emitted 1040
    software packets and ended with a 17.7us ring DRAIN that blocked
    GpSimd until ~27us - killing it frees GpSimd for the k-sum chain.
  - TensorE transposes the summed kernel (8 [128,128] fp16 tiles/band)
    and runs the contraction; the post-k-sum stage is emitted one
    iteration late (software pipelining) so no cross-band serialized
    loop forms across engine queues.
  - The 1/sum(coefs) normalization rides free on the ScalarE PSUM->SBUF
    output copy (activation scale). fp16 keeps total quantization error
    ~6e-4 << the 2e-2 gate.
"""

import sys

if "/opt/trn_rl_repo" not in sys.path:
    sys.path.insert(0, "/opt/trn_rl_repo")

import numpy as np

import concourse.bass as bass
import concourse.mybir as mybir
from concourse import tile
from concourse.bacc import Bacc
from concourse.masks import make_identity
from concourse.bass_utils import run_bass_kernel_spmd

# Problem shapes (hardcoded per the harness contract).
B, K, N, D = 8, 9, 1024, 64
EPS = 0.01
P = 128          # SBUF partitions
NT = N // P      # 8 row/col tiles of the [N, N] kernel

F32 = mybir.dt.float32
F16 = mybir.dt.float16

R = 1.0 - EPS                                  # coefficient ratio
S = float(sum(R ** k for k in range(K)))       # sum of coefs

# The fused scalar_tensor_tensor op is DVE-only (core v3 ISA rejects
# TensorScalarPtr on Pool), so the whole k-sum chain runs on VectorE:
# 9 fused ops x ~0.7us = ~6.3us/band, well under the ~11.5us stream cadence.
VC = N

# Slab groups per band-DMA, in issue order. The chain consumes groups in
# this order, so put the high-k slabs (chain start) in the first DMA.
GROUPS = [(6, 9), (3, 6), (0, 3)]


def build_bass() -> bass.Bass:
    # Bacc (not plain Bass): its compile() runs generate_event_semaphores /
    # move_matmul_waits_to_ldweights, splitting multi-semaphore waits that
    # the 64B ISA instructions (single EVENTS slot) cannot carry.
    nc = Bacc()

    x_d = nc.dram_tensor("x_b", (N, D), F32, kind="ExternalInput")
    g_d = nc.dram_tensor("g_b", (K, N, N), F32, kind="ExternalInput")
    o_d = nc.dram_tensor("out_b", (N, D), F32, kind="ExternalOutput")

    MULT = mybir.AluOpType.mult
    ADD = mybir.AluOpType.add

    with tile.TileContext(nc) as tc:
        with (
            tc.tile_pool(name="consts", bufs=1) as consts,
            tc.tile_pool(name="gr", bufs=3) as gr_pool,
            tc.tile_pool(name="wk", bufs=3) as wk_pool,
            tc.tile_pool(name="wkt", bufs=2) as wkt_pool,
            tc.tile_pool(name="outp", bufs=2) as out_pool,
            tc.tile_pool(name="ps_t", bufs=4, space=bass.MemorySpace.PSUM) as ps_t,
            tc.tile_pool(name="ps_e", bufs=2, space=bass.MemorySpace.PSUM) as ps_e,
        ):
            g_r = g_d.rearrange("k (b p) n -> b p k n", p=P)

            def load_band(i):
                """3 HWDGE DMAs of 3 slabs each; returns {k: column-slice}."""
                tiles = {}
                for gi, (k0, k1) in enumerate(GROUPS):
                    t = gr_pool.tile([P, (k1 - k0) * N], F32, tag=f"g{gi}",
                                     name=f"g{gi}")
                    nc.sync.dma_start(t[:], g_r[i, :, k0:k1, :])
                    for k in range(k0, k1):
                        tiles[k] = t[:, (k - k0) * N : (k - k0 + 1) * N]
                return tiles

            band_tiles = load_band(0)

            # x: f32 on the ACT HWDGE ring (NOT gpsimd SWDGE: the software
            # descriptor path emitted 1040 packets + a 17.7us ring drain
            # that pinned GpSimd until ~27us), then one ACT cast to fp16.
            # Layout [p, chunk, d]: chunk c is a [128, 64] tile with the
            # contraction index m = c*128 + p on partitions.
            x_f32 = consts.tile([P, NT, D], F32)
            nc.scalar.dma_start(x_f32[:], x_d.rearrange("(c p) d -> p c d", p=P))
            x_sb = consts.tile([P, NT, D], F16)
            nc.scalar.activation(
                x_sb[:], x_f32[:], mybir.ActivationFunctionType.Copy, scale=1.0
            )

            # fp16 identity for TensorE transpose. Built by GPSIMD (memset +
            # affine_select, ~0.5us), then copied through VectorE so the
            # first PE transpose waits on a single semaphore (DVE) -
            # Matmult lowering only supports one sync wait.
            ident_raw = consts.tile([P, P], F16)
            make_identity(nc, ident_raw)
            ident = consts.tile([P, P], F16)
            nc.vector.tensor_copy(ident[:], ident_raw[:])

            # Post-k-sum pipeline for one band: transposes, wkT staging,
            # contraction, output. Emitted one iteration LATE (software
            # pipelining) so no cross-band serialized loop forms.
            def emit_tail(i, wk):
                wkT_sb = wkt_pool.tile([P, NT, P], F16, name="wkT_sb")
                for c in range(NT):
                    wkT_ps = ps_t.tile([P, P], F16, name="wkT_ps")
                    nc.tensor.transpose(wkT_ps[:], wk[:, c * P : (c + 1) * P], ident[:])
                    nc.scalar.copy(wkT_sb[:, c, :], wkT_ps[:])

                # emb[band i] = sum_c wk_tile(i,c) @ x_chunk(c), accumulated
                # in PSUM (fp32) over the 8 contraction chunks.
                emb_ps = ps_e.tile([P, D], F32, name="emb_ps")
                for c in range(NT):
                    nc.tensor.matmul(
                        emb_ps[:],
                        wkT_sb[:, c, :],
                        x_sb[:, c, :],
                        start=(c == 0),
                        stop=(c == NT - 1),
                    )

                # PSUM -> SBUF with the 1/S normalization folded into the
                # ACT copy's free scale.
                o_sb = out_pool.tile([P, D], F32, name="o_sb")
                nc.scalar.activation(
                    o_sb[:], emb_ps[:], mybir.ActivationFunctionType.Copy,
                    scale=1.0 / S,
                )
                # Output DMA on the ACT HWDGE ring, NOT sync: an out-DMA on
                # the sync FIFO would block the f32 slab loads queued behind
                # it until this band's whole pipeline finishes.
                nc.scalar.dma_start(o_d[i * P : (i + 1) * P, :], o_sb[:])

            # k consumption order = DMA arrival order (group 0 first).
            K_ORDER = [k for (k0, k1) in GROUPS for k in range(k1 - 1, k0 - 1, -1)]

            pending = None
            for i in range(NT):
                g_ts = band_tiles
                if i + 1 < NT:
                    band_tiles = load_band(i + 1)

                # Weighted k-sum, wk = sum_k r^k * g_k, accumulated in fp16
                # via fused scalar_tensor_tensor: wk = (g_k * c_k) + wk.
                # Two independent serial chains: VectorE on cols [0:VC),
                # GpSimd on cols [VC:N).
                wk = wk_pool.tile([P, N], F16, name="wk")
                halves = [(nc.vector, slice(0, VC))]
                if VC < N:
                    halves.append((nc.gpsimd, slice(VC, N)))
                for eng, cols in halves:
                    k_init = K_ORDER[0]
                    eng.tensor_scalar_mul(
                        wk[:, cols], g_ts[k_init][:, cols], R ** k_init
                    )
                    for k in K_ORDER[1:]:
                        eng.scalar_tensor_tensor(
                            wk[:, cols], g_ts[k][:, cols], R ** k, wk[:, cols],
                            MULT, ADD,
                        )

                if pending is not None:
                    emit_tail(*pending)
                pending = (i, wk)

            emit_tail(*pending)

    nc.compile()
    return nc


_NC = None


def _get_nc() -> bass.Bass:
    global _NC
    if _NC is None:
        _NC = build_bass()
    return _NC


def run(x: np.ndarray, gr_kernel: np.ndarray, **spmd_kwargs):
    """Run the SPMD kernel on cores 0-7; returns BassKernelResults."""
    nc = _get_nc()
    in_maps = [
        {
            "x_b": np.ascontiguousarray(x[b], dtype=np.float32),
            "g_b": np.ascontiguousarray(gr_kernel[b], dtype=np.float32),
        }
        for b in range(B)
    ]
    return run_bass_kernel_spmd(nc, in_maps, core_ids=list(range(B)), **spmd_kwargs)


def kernel(x: np.ndarray, gr_kernel: np.ndarray) -> np.ndarray:
    res = run(np.asarray(x), np.asarray(gr_kernel))
    out = np.stack([res.results[b]["out_b"] for b in range(B)], axis=0)
    return out.astype(np.float32, copy=False)


if __name__ == "__main__":
    rng = np.random.default_rng(0)
    x = rng.standard_normal((B, N, D), dtype=np.float32)
    g = rng.standard_normal((B, K, N, N), dtype=np.float32)
    out = kernel(x, g)
    coefs = (1.0 - EPS) ** np.arange(K)
    wk = np.einsum("k,bknm->bnm", coefs, g)
    ref = np.matmul(wk, x) / coefs.sum()
    err = np.linalg.norm(out - ref) / np.linalg.norm(ref)
    print("self-check rel err:", err)
